# revision 11
# baseline (speedup 1.0000x reference)
"""Multi-head attention block (nn_AttentionBlock) on 8 Trainium2 NeuronCores.

Reference computation (fp32):
    qkv = x @ w_qkv;  q,k,v = split(qkv)
    per head: att = softmax(q @ k.T / 8) @ v
    out = concat_heads(att) @ w_out
Shapes: x [4, 2048, 1024], w_qkv [1024, 3072], w_out [1024, 1024], 16 heads.

Sharding: batch x head-half. Core c handles batch b = c//2 and heads
h0 = (c%2)*8 .. h0+8. Each core computes its 8 heads' attention and a partial
output projection (row-shard of w_out); host sums the two partials per batch.

Per-core kernel (all heavy matmuls bf16 with fp32 PSUM accumulation):
  P1: qkT[n, s] = w_qk.T @ x.T (weights stationary, xT moving) -> q/k heads
      transposed in SBUF; v[t, d] = x @ w_v in natural layout with an
      appended ones column per head (v_aug).
  P2 per head pair (2 heads share a 128-partition tile, K=64 row-packed
      matmuls at base partitions 0/64):
      ST[t, s] = exp((k_t . q_s)/8)  via TensorE + ScalarE exp (psum->sbuf
      bf16), no max subtraction (logits are O(5), exp is safe in fp32).
      PV: att_aug[d|1, s] = v_aug.T @ p accumulated over 16 t-chunks; row 64
      is r[s] = sum_t p. rinv broadcast via K=1 ones matmul + reciprocal;
      normalize att rows with a tensor-tensor multiply.
  P3: partial[s, e] = att.T @ w_out accumulated over the 4 head pairs.
"""

import sys

sys.path.insert(0, "/opt/trn_rl_repo")

import numpy as np
import ml_dtypes

import concourse.mybir as mybir
import concourse.tile as tile
from concourse import bacc
from concourse import bass_utils

F32 = mybir.dt.float32
F32R = mybir.dt.float32r
BF16 = mybir.dt.bfloat16

S = 2048          # sequence length
D = 1024          # embed dim
DH = 64           # head dim
NH = 8            # heads per core
NPAIR = 4         # head pairs per core
KC = D // 128     # contraction chunks for phase 1
SC = S // 512     # 512-wide free chunks of the sequence
TT = S // 128     # 128-row t tiles of the sequence
N_CORES = 8

_CACHED_NC = None


def build_nc():
    nc = bacc.Bacc("TRN2", target_bir_lowering=False, debug=False,
                   num_devices=N_CORES)

    xt = nc.dram_tensor("xt", [D, S], BF16, kind="ExternalInput").ap()
    wqk = nc.dram_tensor("wqk", [D, D], BF16, kind="ExternalInput").ap()
    wv = nc.dram_tensor("wv", [D, NH * DH], BF16, kind="ExternalInput").ap()
    wout = nc.dram_tensor("wout", [NH * DH, D], BF16, kind="ExternalInput").ap()
    out_p = nc.dram_tensor("out_p", [S, D], F32, kind="ExternalOutput").ap()

    with tile.TileContext(nc) as tc:
        with (
            tc.tile_pool(name="sbc", bufs=1) as sbc,        # constants
            tc.tile_pool(name="sbin", bufs=1) as sbin,      # phase-1 inputs
            tc.tile_pool(name="sbqk", bufs=1) as sbqk,      # qkT persistent
            tc.tile_pool(name="sbv", bufs=1) as sbv,        # v_aug persistent
            tc.tile_pool(name="sbp", bufs=8) as sbp,        # exp(ST) tiles
            tc.tile_pool(name="sbr", bufs=4) as sbr,        # r chunks (f32r)
            tc.tile_pool(name="sbrv", bufs=2) as sbrv,      # rinv chunks
            tc.tile_pool(name="sbst", bufs=2) as sbst,      # B-head att stage
            tc.tile_pool(name="sbat", bufs=1) as sbat,      # att pairs
            tc.tile_pool(name="sbo", bufs=2) as sbo,        # out evac
            tc.tile_pool(name="psb", bufs=2, space="PSUM") as psb,   # [128,1024]
            tc.tile_pool(name="psa", bufs=4, space="PSUM") as psa,   # [128,512]
        ):
            # ---------- constants ----------
            ones_f = sbc.tile([128, 128], F32, name="ones_f")
            nc.gpsimd.memset(ones_f[:], 1.0)
            ones_r = sbc.tile([128, 128], F32R, name="ones_r")
            nc.vector.tensor_copy(ones_r[:], ones_f[:])

            # ---------- input DMA ----------
            xt_t = [sbin.tile([128, S], BF16, name=f"xt{k}", tag=f"xt{k}")
                    for k in range(KC)]
            wqk_t = [sbin.tile([128, D], BF16, name=f"wqk{k}", tag=f"wqk{k}")
                     for k in range(KC)]
            wv_t = [sbin.tile([128, NH * DH], BF16, name=f"wv{k}", tag=f"wv{k}")
                    for k in range(KC)]
            wout_t = [sbin.tile([128, D], BF16, name=f"wout{p}", tag=f"wout{p}")
                      for p in range(NPAIR)]
            for k in range(KC):
                nc.sync.dma_start(xt_t[k][:], xt[k * 128:(k + 1) * 128, :])
                nc.sync.dma_start(wqk_t[k][:], wqk[k * 128:(k + 1) * 128, :])
                nc.sync.dma_start(wv_t[k][:], wv[k * 128:(k + 1) * 128, :])
            for p in range(NPAIR):
                nc.sync.dma_start(wout_t[p][:], wout[p * 128:(p + 1) * 128, :])

            # ---------- persistent intermediates ----------
            # qkT rows: tiles 0..3 = qT head pairs, 4..7 = kT head pairs
            qk_t = [sbqk.tile([128, S], BF16, name=f"qk{n}", tag=f"qk{n}")
                    for n in range(2 * NPAIR)]
            # v_aug: [t, 8*(64+1)]; per head 64 v columns then a ones column
            v_t = [sbv.tile([128, NH * (DH + 1)], BF16, name=f"v{t}", tag=f"v{t}")
                   for t in range(TT)]
            att_t = [sbat.tile([128, S], BF16, name=f"att{p}", tag=f"att{p}")
                     for p in range(NPAIR)]

            # ---------- phase 1b: v (natural layout) ----------
            for tb in range(TT):
                ps = psa.tile([128, 512], F32, name=f"psv{tb}", tag="psa")
                for k in range(KC):
                    nc.tensor.matmul(
                        ps[:, 0:NH * DH],
                        xt_t[k][:, tb * 128:(tb + 1) * 128],
                        wv_t[k][:],
                        start=(k == 0),
                        stop=(k == KC - 1),
                    )
                nc.gpsimd.memset(v_t[tb][:], 1.0)
                nc.vector.tensor_copy(
                    v_t[tb][:].rearrange("p (h e) -> p h e", e=DH + 1)[:, :, 0:DH],
                    ps[:, 0:NH * DH].rearrange("p (h e) -> p h e", e=DH),
                )

            # ---------- phase 1a: qT / kT for head pair 0 only ----------
            # The remaining qkT blocks are computed inside the attention
            # stream (extra PE work keeps the tensor engine HAM-warm while
            # ScalarE exp is the bottleneck).
            def emit_p1a_unit(nb, scp):
                """Compute qk_t[nb][:, scp*1024:+1024] (2 s-chunks) in one
                2-bank psb tile."""
                ps = psb.tile([128, 1024], F32, name=f"psq{nb}_{scp}",
                              tag="psb")
                for half1 in range(2):
                    sc = 2 * scp + half1
                    for k in range(KC):
                        nc.tensor.matmul(
                            ps[:, half1 * 512:(half1 + 1) * 512],
                            wqk_t[k][:, nb * 128:(nb + 1) * 128],
                            xt_t[k][:, sc * 512:(sc + 1) * 512],
                            start=(k == 0),
                            stop=(k == KC - 1),
                        )
                nc.vector.tensor_copy(
                    qk_t[nb][:, scp * 1024:(scp + 1) * 1024], ps[:])

            for pr in range(NPAIR):
                for nb in (pr, NPAIR + pr):
                    for scp in range(SC // 2):
                        emit_p1a_unit(nb, scp)

            # ---------- phase 2: attention, software-pipelined ----------
            # Per head: ST matmuls feed exp (ACT is the steady bottleneck);
            # PV matmuls are interleaved into the ST stream with a 2-tile lag
            # so the PE always has backlog (keeps HAM at 2.4 GHz). The
            # normalization tail of head h-1 is emitted inside head h's ST
            # phase so the PE never sees a multi-us idle window.
            PVLAG = 2

            def emit_r_copies(pr, half, acc):
                """Round the PV ones-row sums (r) to f32r right after the last
                PV matmul so the broadcast matmuls can fire early."""
                r_c = []
                for c in range(SC):
                    r = sbr.tile([128, 512], F32R,
                                 name=f"r{pr}_{half}_{c}", tag="r")
                    nc.vector.tensor_copy(r[64:65, :], acc[c][64:65, :])
                    r_c.append(r)
                return r_c

            def make_tail(pr, half, acc, r_c):
                """Normalization tail for a finished head: broadcast 1/r and
                scale; emitted later, inside the next head's ST stream."""
                def tail():
                    if half == 0:
                        dst = att_t[pr]
                    else:
                        dst = sbst.tile([64, S], BF16, name=f"bst{pr}",
                                        tag="bstage")
                    for i in range(2):
                        rb = psb.tile([128, 1024], F32,
                                      name=f"rb{pr}_{half}_{i}", tag="psb")
                        for c in (2 * i, 2 * i + 1):
                            nc.tensor.matmul(
                                rb[0:64, (c % 2) * 512:(c % 2 + 1) * 512],
                                ones_r[64:65, 0:64],
                                r_c[c][64:65, :],
                                start=True,
                                stop=True,
                            )
                        rinv = sbrv.tile([64, 1024], F32,
                                         name=f"rinv{pr}_{half}_{i}",
                                         tag="rinv")
                        nc.vector.reciprocal_approx_fast(rinv[:], rb[0:64, :])
                        for c in (2 * i, 2 * i + 1):
                            nc.vector.tensor_mul(
                                dst[0:64, c * 512:(c + 1) * 512],
                                acc[c][0:64, :],
                                rinv[:, (c % 2) * 512:(c % 2 + 1) * 512],
                            )
                    if half == 1:
                        # shift head B rows into partitions 64..128
                        nc.sync.dma_start(att_t[pr][64:128, :], dst[0:64, :])
                return tail

            p_tiles = {}    # h -> [p tile per tt]
            accs = {}       # h -> [4 psum accumulators]
            pending_tail = None

            def emit_st(h, tt):
                pr, half = h // 2, h % 2
                qt, kt = qk_t[pr], qk_t[NPAIR + pr]
                r0, r1 = half * 64, half * 64 + 64
                for sh in range(2):
                    ps = psb.tile([128, 1024], F32,
                                  name=f"pst{h}_{tt}_{sh}", tag="psb")
                    for j in range(2):
                        nc.tensor.matmul(
                            ps[:, j * 512:(j + 1) * 512],
                            kt[r0:r1, tt * 128:(tt + 1) * 128],
                            qt[r0:r1, sh * 1024 + j * 512:
                               sh * 1024 + (j + 1) * 512],
                            start=True,
                            stop=True,
                        )
                    nc.scalar.activation(
                        p_tiles[h][tt][:, sh * 1024:(sh + 1) * 1024],
                        ps[:],
                        mybir.ActivationFunctionType.Exp,
                        scale=0.125,
                    )

            def emit_pv(h, tt):
                for c in range(SC):
                    nc.tensor.matmul(
                        accs[h][c][0:DH + 1, :],
                        v_t[tt][:, h * (DH + 1):(h + 1) * (DH + 1)],
                        p_tiles[h][tt][:, c * 512:(c + 1) * 512],
                        start=(tt == 0),
                        stop=(tt == TT - 1),
                    )

            seq = [(h, tt) for h in range(NH) for tt in range(TT)]
            for i, (h, tt) in enumerate(seq):
                if tt == 0:
                    p_tiles[h] = [
                        sbp.tile([128, S], BF16, name=f"p{h}_{tt2}", tag="p")
                        for tt2 in range(TT)]
                emit_st(h, tt)
                if tt == PVLAG:
                    # retire head h-1 (frees its psa accumulators), then
                    # open this head's accumulators
                    if pending_tail is not None:
                        pending_tail()
                        pending_tail = None
                    accs[h] = [psa.tile([128, 512], F32,
                                        name=f"pv{h}_{c}", tag="psa")
                               for c in range(SC)]
                if i >= PVLAG:
                    hp, ttp = seq[i - PVLAG]
                    emit_pv(hp, ttp)
                    if ttp == TT - 1:
                        r_c = emit_r_copies(hp // 2, hp % 2, accs[hp])
                        pending_tail = make_tail(hp // 2, hp % 2, accs[hp], r_c)
            for i in range(len(seq) - PVLAG, len(seq)):
                hp, ttp = seq[i]
                emit_pv(hp, ttp)
                if ttp == TT - 1:
                    r_c = emit_r_copies(hp // 2, hp % 2, accs[hp])
                    pending_tail = make_tail(hp // 2, hp % 2, accs[hp], r_c)
            # retire the last head before the output projection
            pending_tail()

            # ---------- phase 3: output projection ----------
            for sb_i in range(TT):
                for e in range(2):
                    ps = psa.tile([128, 512], F32, name=f"po{sb_i}_{e}",
                                  tag="psa")
                    for pr in range(NPAIR):
                        nc.tensor.matmul(
                            ps[:],
                            att_t[pr][:, sb_i * 128:(sb_i + 1) * 128],
                            wout_t[pr][:, e * 512:(e + 1) * 512],
                            start=(pr == 0),
                            stop=(pr == NPAIR - 1),
                        )
                    o = sbo.tile([128, 512], F32, name=f"o{sb_i}_{e}", tag="o")
                    nc.vector.tensor_copy(o[:], ps[:])
                    nc.sync.dma_start(
                        out_p[sb_i * 128:(sb_i + 1) * 128,
                              e * 512:(e + 1) * 512], o[:])

    nc.compile()
    return nc


def get_nc():
    global _CACHED_NC
    if _CACHED_NC is None:
        _CACHED_NC = build_nc()
    return _CACHED_NC


def make_in_maps(x, w_qkv, w_out):
    bf = ml_dtypes.bfloat16
    in_maps = []
    for c in range(N_CORES):
        b = c // 2
        h0 = (c % 2) * NH
        q_cols = w_qkv[:, h0 * DH:(h0 + NH) * DH]
        k_cols = w_qkv[:, D + h0 * DH:D + (h0 + NH) * DH]
        v_cols = w_qkv[:, 2 * D + h0 * DH:2 * D + (h0 + NH) * DH]
        in_maps.append({
            "xt": np.ascontiguousarray(x[b].T).astype(bf),
            "wqk": np.concatenate([q_cols, k_cols], axis=1).astype(bf),
            "wv": np.ascontiguousarray(v_cols).astype(bf),
            "wout": np.ascontiguousarray(
                w_out[h0 * DH:(h0 + NH) * DH, :]).astype(bf),
        })
    return in_maps


def run(x, w_qkv, w_out, trace=False, trace_cores=None):
    nc = get_nc()
    in_maps = make_in_maps(x, w_qkv, w_out)
    res = bass_utils.run_bass_kernel_spmd(
        nc, in_maps, core_ids=list(range(N_CORES)),
        trace=trace, trace_cores=trace_cores,
    )
    partials = [res.results[c]["out_p"] for c in range(N_CORES)]
    out = np.stack([partials[2 * b] + partials[2 * b + 1] for b in range(4)])
    return out.astype(np.float32), res


def kernel(x, w_qkv, w_out):
    out, _ = run(np.asarray(x), np.asarray(w_qkv), np.asarray(w_out))
    return out


if __name__ == "__main__":
    import tempfile
    nc = build_nc()
    with tempfile.TemporaryDirectory() as td:
        neff = bass_utils.compile_bass_kernel(nc, td)
        print("LOCAL COMPILE OK:", neff)
